# revision 21
# baseline (speedup 1.0000x reference)
"""Trainium2 Bass kernel for BoT-style attention (nn_Attention_20968030339767).

Data-parallel over batch: 16 batches -> 2 per NeuronCore, 8 cores, no
collectives.  All BN folding / bias-table exponentiation happens on host;
the device runs projections + attention + hardswish + output projection.

Math (per batch b):
  q = BN(Wq x), k = BN(Wk x), v = BN(Wv x)          (1x1 conv == channel matmul)
  logits = SCALE*(q.k) + emb[pos]/SCALE
  attn   = softmax(logits); out = attn @ v
  hs     = hardswish(out); y = BN(W_out hs + b_out)

Device-side formulation (per core, all matmuls bf16, PSUM f32):
  qT = Wq'' x   [512, 1024]  (Wq'' = SCALE*diag(sq)*Wq; bias bq'' at evac)
  kT = Wk'  x   [512, 1024]
  vT = x^T Wv'^T + bv'       [1024, 8, 65]  (per-head blocks: 64 v-cols + ones)

  Attention runs per (head-pair p, batch b, i-half ic).  The channel layout
  puts head 2p at SBUF partitions 0-63 and head 2p+1 at 64-127 of the m2=p
  projection tile, so the two heads' QK^T matmuls (K=64 contraction) run
  CONCURRENTLY in the PE array via row-group tiling (tile_position derives
  from base_partition 0/64), writing one 2-bank PSUM tile
  st2 [128 j, 1024] = [h (512 i) | h' (512 i)] per (j, ic).  Then per j-pair:
    exp(st2_j), exp(st2_j+1) -> halves of one p2 [128, 2048] bf16 tile
    p2 *= EB4[p, jp, ic]         one in-place DVE instr, 2x bf16 mode
    utpair[:, 0:512]    += vT_aug_h^T  @ p2-slices   (row 64 = sumexp)
    utpair[:, 512:1024] += vT_aug_h'^T @ p2-slices
  since vT carries bv': UT[0:64] = U_raw + bv'*sumexp.
  Normalize chain per ic (software-pipelined: emitted after the NEXT ic's
  first j-pair so the FIFO engine queues never stall on it):
    sums copy -> reciprocal_approx_fast (~51 ULP, plenty) -> DRAM round-trip
    broadcast f32 [64, N] -> u2 quadrant = UT[0:64] * bcast  (reads PSUM,
    evacuates the UT banks; = U/sum + bv' after normalize)
  hardswish in 2 DVE ops: t1 = max(u2+3, 0);  hs = min(t1, 6) * u2
  (the /6 is folded into Wo'')
  y = Wo'' hs + bo''   (Wo'' = diag(so)*W_out/6)

Scheduling notes (the exp chain is the ~134us floor; everything is built to
keep the scalar engine streaming):
  - PSUM (8 banks exactly): tag "st2" [128,1024] bufs=2 (4 banks) is
    attention-EXCLUSIVE so no projection tile ever FIFO-chains ahead of an
    exp; tag "ut" bufs=2 (4 banks) holds the utpair accumulators plus the
    qp/kp/vp/yp projection tiles (PV-side stalls absorb via the deep p2 ring).
  - PV matmuls are emitted TWO j-pairs late via a rolling queue that
    crosses ic/unit boundaries: at ~94% vector utilization each eb-multiply
    lands ~1 slot late, so lag-1 PVs were mul-paced and dragged the next QKs
    (and thus exps) with them through the in-order tensor queue.
  - Projections for pair p+1 and its EB fetch are emitted one unit early,
    woven between attention units.
  - gpsimd is useless here: ~2.9us/instr tensor ops and no PSUM port.

Softmax max-subtraction is skipped: |logits| <= ~7 here, exp is safe in f32.
No collectives; cores are fully independent (batch-parallel).
"""

import sys
import functools

import numpy as np

sys.path.insert(0, "/opt/trn_rl_repo")

import ml_dtypes  # noqa: E402

from concourse import bacc, mybir  # noqa: E402
import concourse.bass as bass  # noqa: E402
import concourse.tile as tile  # noqa: E402
from concourse.bass_utils import run_bass_kernel_spmd  # noqa: E402

BF16 = mybir.dt.bfloat16
F32 = mybir.dt.float32
Af = mybir.ActivationFunctionType
Op = mybir.AluOpType

B, DIM, H, DK, N = 16, 256, 8, 64, 1024
INK = H * DK  # 512
SCALE = DK ** -0.5
EPS = 1e-5
NCORES = 8
BPC = B // NCORES  # batches per core
NPAIR = H // 2  # head pairs
# which jp's eb-multiply runs on gpsimd instead of the (bottleneck) DVE.
# jp3 must stay on the DVE: its PV pops right before the part1 drain, so it
# cannot take the extra lag slot that slow (~4.4us) gpsimd multiplies need.
EB_GP_JP = frozenset({1, 2})


def build_body(nc, tc, d):
    """Emit the whole per-core program inside a TileContext."""
    ts = bass.ts

    _n = [0]
    pool = d["_pool"]

    def T(shape, dtype, **kw):
        if "name" not in kw:
            kw["name"] = f"{kw.get('tag', 't')}_{_n[0]}"
            _n[0] += 1
        space = kw.pop("space", None)
        if space == "PSUM":
            p = d["_psum_pool"]
        elif space == "DRAM":
            p = d["_dram_pool"]
        else:
            p = pool
        return p.tile(shape, dtype, **kw)

    # ---- persistent SBUF tensors (weights packed into one DMA) -----------
    wpk = T([128, 4096], BF16, tag="wpk", bufs=1)
    bpk = T([128, 10], F32, tag="bpk", bufs=1)
    bv_bc = T([128, 512], BF16, tag="bv_bc", bufs=1)
    x_s = []
    for b in range(BPC):
        xt = T([128, 2, N], BF16, tag="x", bufs=BPC, name=f"x{b}")
        nc.sync.dma_start(xt[:], d["x"][b].rearrange("k p n -> p k n"))
        x_s.append(xt)
    # wq/wk first so the first q/k projection matmuls start ASAP
    nc.sync.dma_start(wpk[:, 0:2048], d["wpack"][:, 0:2048])
    nc.sync.dma_start(wpk[:, 2048:3072], d["wpack"][:, 2048:3072])
    nc.sync.dma_start(wpk[:, 3072:4096], d["wpack"][:, 3072:4096])
    nc.sync.dma_start(bpk[:], d["bpack"][:])
    # bv broadcast to all 128 partitions (token rows) via step-0 DMA
    nc.sync.dma_start(bv_bc[:], d["bv"].broadcast_to([128, 512]))
    wq_s = wpk[:].rearrange("p (w k o) -> p w k o", w=4, k=2)[:, 0]
    wk_s = wpk[:].rearrange("p (w k o) -> p w k o", w=4, k=2)[:, 1]
    wv_s = wpk[:].rearrange("p (w k o) -> p w k o", w=4, k=2)[:, 2]
    wo_s = wpk[:, 3072:4096].rearrange("p (k o) -> p k o", k=4)
    bq_s = bpk[:, 0:4]
    bk_s = bpk[:, 4:8]
    bo_s = bpk[:, 8:10]

    q_s = [[None] * 4 for _ in range(BPC)]
    k_s = [[None] * 4 for _ in range(BPC)]
    v_s = [[None] * 8 for _ in range(BPC)]
    hs_s = [[None] * 4 for _ in range(BPC)]

    # ---- projection pieces (emitted interleaved between attention units) --
    def proj_q(b, m):
        qp = T([128, N], F32, space="PSUM", tag="ut", bufs=2, name=f"qp{b}{m}")
        for ic in range(2):
            for kc in range(2):
                nc.tensor.matmul(
                    qp[:, ts(ic, 512)],
                    wq_s[:, kc, ts(m, 128)],
                    x_s[b][:, kc, ts(ic, 512)],
                    start=(kc == 0),
                    stop=(kc == 1),
                )
        qt = T([128, N], BF16, tag="q", bufs=6, name=f"q{b}{m}")
        nc.vector.tensor_scalar(qt[:], qp[:], bq_s[:, m : m + 1], None, Op.add)
        q_s[b][m] = qt

    def proj_k(b, m):
        kp = T([128, N], F32, space="PSUM", tag="ut", bufs=2, name=f"kp{b}{m}")
        for ic in range(2):
            for kc in range(2):
                nc.tensor.matmul(
                    kp[:, ts(ic, 512)],
                    wk_s[:, kc, ts(m, 128)],
                    x_s[b][:, kc, ts(ic, 512)],
                    start=(kc == 0),
                    stop=(kc == 1),
                )
        kt = T([128, N], BF16, tag="k", bufs=6, name=f"k{b}{m}")
        nc.vector.tensor_scalar(kt[:], kp[:], bk_s[:, m : m + 1], None, Op.add)
        k_s[b][m] = kt

    def proj_qk(b, m):
        proj_q(b, m)
        proj_k(b, m)

    def proj_v(b):
        for t in range(8):
            vp = T([128, 512], F32, space="PSUM", tag="ut", bufs=2, name=f"vp{b}{t}")
            for kc in range(2):
                nc.tensor.matmul(
                    vp[:],
                    x_s[b][:, kc, ts(t, 128)],
                    wv_s[:, kc, :],
                    start=(kc == 0),
                    stop=(kc == 1),
                )
            # evac + bv'' add; bias rides into UT as bv*sumexp
            vt = T([128, 8, 65], BF16, tag="v", bufs=8 * BPC, name=f"v{b}{t}")
            nc.gpsimd.memset(vt[:, :, 64:65], 1.0)
            nc.vector.tensor_add(
                vt[:, :, 0:64],
                vp[:].rearrange("p (h e) -> p h e", e=64),
                bv_bc[:].rearrange("p (h e) -> p h e", e=64),
            )
            v_s[b][t] = vt

    def yproj(b):
        for m in range(2):
            for t2 in range(2):
                yp = T([128, 512], F32, space="PSUM", tag="ut", bufs=2,
                       name=f"yp{b}{m}{t2}")
                for kc in range(4):
                    nc.tensor.matmul(
                        yp[:],
                        wo_s[:, kc, ts(m, 128)],
                        hs_s[b][kc][:, ts(t2, 512)],
                        start=(kc == 0),
                        stop=(kc == 3),
                    )
                ys = T([128, 512], F32, tag="y", bufs=3, name=f"ys{b}{m}{t2}")
                nc.scalar.activation(ys[:], yp[:], Af.Identity, bias=bo_s[:, m : m + 1])
                nc.sync.dma_start(d["y"][b, m, :, ts(t2, 512)], ys[:])

    # ---- attention unit: head-pair (2p, 2p+1) of batch b ------------------
    # The recip/normalize chain for each ic-half is emitted AFTER the next
    # ic's first j-pair so the FIFO engine queues never stall on it.
    pend = [None, None]
    pvq = []

    def drain(slot):
        if pend[slot] is not None:
            pend[slot]()
            pend[slot] = None

    def unit(p, b, ebt, pieces=(), ebq=()):
        u2 = T([128, N], BF16, tag="u2", bufs=2, name=f"u2{p}{b}")

        def mk_chain(ut, ic, final):
            # split in two so the vector-FIFO insert is spread over two drain
            # slots (and the broadcast DMA has landed before part2 needs it)
            bcast = T([64, N], F32, tag="bcast", bufs=2, name=f"bc{p}{b}{ic}")

            def part1():
                # softmax denominators: UT row 64 = [sum_h | sum_h'].
                # approx-recip (~51 ULP, plenty) straight off the PSUM sums
                # row, then a DRAM round-trip broadcast to [64, N] f32 (DMA
                # engines are idle; gpsimd is reserved for eb-multiplies).
                sums = T([1, N], F32, tag="sums", bufs=1, name=f"sm{p}{b}{ic}")
                nc.vector.tensor_copy(sums[:], ut[64:65, :])
                rc = T([1, N], F32, tag="rc", bufs=1, name=f"rc{p}{b}{ic}")
                nc.vector.reciprocal_approx_fast(rc[:], sums[:])
                dr = T([1, N], F32, space="DRAM", tag="dr", bufs=2,
                       name=f"dr{p}{b}{ic}")
                nc.sync.dma_start(dr[:], rc[:])
                nc.sync.dma_start(bcast[:], dr[:].broadcast_to([64, N]))

            def part2():
                # normalize straight out of PSUM (also evacuates the UT banks)
                nc.vector.tensor_mul(
                    u2[0:64, ts(ic, 512)], ut[0:64, 0:512], bcast[:, 0:512]
                )
                nc.vector.tensor_mul(
                    u2[64:128, ts(ic, 512)], ut[0:64, 512:1024], bcast[:, 512:1024]
                )
                if final:
                    # hardswish: hs = min(max(u2+3,0),6) * u2  (/6 in Wo'')
                    t1 = T([128, N], BF16, tag="t1", bufs=2, name=f"t1{p}{b}")
                    nc.vector.tensor_scalar(t1[:], u2[:], 3.0, 0.0, Op.add, Op.max)
                    hst = T([128, N], BF16, tag="hs", bufs=4 * BPC, name=f"hs{p}{b}")
                    nc.vector.scalar_tensor_tensor(
                        hst[:], t1[:], 6.0, u2[:], Op.min, Op.mult
                    )
                    hs_s[b][p] = hst
            return part1, part2

        for ic in range(2):
            ut = T([65, N], F32, space="PSUM", tag="ut", bufs=2, name=f"ut{p}{b}{ic}")

            def pv(jp, p2, ut=ut):
                for jh in range(2):
                    j = 2 * jp + jh
                    nc.tensor.matmul(
                        ut[:, 0:512],
                        v_s[b][j][:, 2 * p, :],
                        p2[:, 1024 * jh : 1024 * jh + 512],
                        start=(j == 0),
                        stop=(j == 7),
                    )
                    nc.tensor.matmul(
                        ut[:, 512:1024],
                        v_s[b][j][:, 2 * p + 1, :],
                        p2[:, 1024 * jh + 512 : 1024 * jh + 1024],
                        start=(j == 0),
                        stop=(j == 7),
                    )

            for jp in range(4):
                # j-pair shares one [128, 2048] p2 (exp writes halves, eb-mul
                # in place) so the multiply runs as one DVE instr in 2x mode;
                # PV for pair jp is emitted one pair late via a rolling queue
                # that crosses ic/unit boundaries, so neither a multiply nor a
                # PV burst ever stalls the next QKs (in-order tensor queue)
                p2 = T([128, 2048], BF16, tag="ep", bufs=10, name=f"ep{p}{b}{ic}{jp}")
                for jh in range(2):
                    j = 2 * jp + jh
                    st2 = T([128, N], F32, space="PSUM", tag="st2", bufs=2,
                            name=f"st{p}{b}{ic}{j}")
                    # the two heads' QK^T run concurrently in the PE array
                    # (row groups 0/64 via base_partition -> different banks)
                    nc.tensor.matmul(
                        st2[:, 0:512],
                        k_s[b][p][0:64, ts(j, 128)],
                        q_s[b][p][0:64, ts(ic, 512)],
                        start=True,
                        stop=True,
                    )
                    nc.tensor.matmul(
                        st2[:, 512:1024],
                        k_s[b][p][64:128, ts(j, 128)],
                        q_s[b][p][64:128, ts(ic, 512)],
                        start=True,
                        stop=True,
                    )
                    nc.scalar.activation(p2[:, ts(jh, 1024)], st2[:], Af.Exp)
                # PV lag is TWO pairs (THREE for gpsimd-multiplied pairs: the
                # gp multiply takes ~4.4us vs ~1.2 on DVE, so its PVs get one
                # extra emission slot of slack, deferred at most once).  At
                # jp==1 ALL of the previous ic's PVs are force-flushed so the
                # part1 drain never reads the sums row before its final
                # accumulation has been emitted.
                if jp == 1:
                    while len(pvq) > 1:
                        f, a, t, _ = pvq.pop(0)
                        f(a, t)
                    drain(0)
                elif len(pvq) >= 2:
                    if pvq[0][1] in EB_GP_JP and len(pvq) == 2 and not pvq[0][3]:
                        pvq[0][3] = True  # defer this gp-paced PV once
                    else:
                        f, a, t, _ = pvq.pop(0)
                        f(a, t)
                if jp == 3:
                    # ~2.3us after part1: its broadcast has landed; the u-muls
                    # must beat PVs two blocks out to the UT banks
                    drain(1)
                # eb-multiply engine split: gpsimd (otherwise idle) takes a
                # subset of the 64 multiplies to relieve the DVE bottleneck
                if jp in EB_GP_JP:
                    nc.gpsimd.tensor_mul(p2[:], p2[:], ebt[:, jp, ic, :])
                else:
                    nc.vector.tensor_mul(p2[:], p2[:], ebt[:, jp, ic, :])
                pvq.append([pv, jp, p2, False])
                if jp == 3 and pieces:
                    # a "ut"-ring slot was just scheduled for release by the
                    # part2 drain above: the next projection piece can cycle
                    # the ring with only ~1us of tensor-head wait
                    pieces.pop(0)()
                if ebq:
                    # next pair's EB table arrives one 512KB chunk per slot so
                    # it never head-of-line blocks the bcast round-trips that
                    # share the sync DMA ring
                    ebq.pop(0)()
            pend[0], pend[1] = mk_chain(ut, ic, final=(ic == 1))

    def fetch_eb(p, chunked=False):
        """Allocate pair p's EB tile; either fetch now or return chunk DMAs."""
        ebt = T([128, 4, 2, 2048], BF16, tag="eb", bufs=2, name=f"eb{p}")
        chunks = [
            (lambda jp=jp, c=c: nc.sync.dma_start(ebt[:, jp, c],
                                                  d["eb4"][p, jp, c]))
            for jp in range(4) for c in range(2)
        ]
        if not chunked:
            for ch in chunks:
                ch()
            chunks = []
        return ebt, chunks

    # ---- interleaved emission --------------------------------------------
    # unit order: (p0,b0),(p0,b1),(p1,b0),... so each pair's EB tile serves
    # both batches; proj pieces are woven in so no PSUM-ring FIFO chains a
    # whole phase ahead of the attention stream.
    ebt, _ = fetch_eb(0)
    proj_qk(0, 0)
    proj_v(0)
    proj_qk(1, 0)
    proj_v(1)
    next_eb = None
    pieces = []
    ebq = []
    for p in range(NPAIR):
        for b in range(BPC):
            if b == 0 and p + 1 < NPAIR:
                # stage pair p+1's projections as 4 pieces popped one-by-one
                # right after each part2 drain (2 "ut"-slot release windows
                # per unit) so proj matmuls never bunch up ahead of QKs
                pieces += [
                    lambda m=p + 1: proj_q(0, m),
                    lambda m=p + 1: proj_k(0, m),
                    lambda m=p + 1: proj_q(1, m),
                    lambda m=p + 1: proj_k(1, m),
                ]
            if b == 1 and p + 1 < NPAIR:
                next_eb, ebq[:] = fetch_eb(p + 1, chunked=True)
            unit(p, b, ebt, pieces, ebq)
        if next_eb is not None:
            ebt = next_eb
            next_eb = None
    while pvq:
        f, a, t, _ = pvq.pop(0)
        f(a, t)
    # batch 0's hardswish tiles were all drained mid-stream, so its output
    # projection is independent of the final unit's pending chain: emit it
    # first to overlap that chain's DMA round-trip
    yproj(0)
    drain(0)
    drain(1)
    yproj(1)


@functools.cache
def build_nc():
    nc = bacc.Bacc(
        "TRN2",
        target_bir_lowering=False,
        debug=False,
        enable_asserts=False,
        num_devices=NCORES,
    )
    d = {
        "x": nc.dram_tensor("x", [BPC, 2, 128, N], BF16, kind="ExternalInput"),
        "wpack": nc.dram_tensor("wpack", [128, 4096], BF16, kind="ExternalInput"),
        "bpack": nc.dram_tensor("bpack", [128, 10], F32, kind="ExternalInput"),
        "bv": nc.dram_tensor("bv", [1, 512], BF16, kind="ExternalInput"),
        "eb4": nc.dram_tensor(
            "eb4", [NPAIR, 4, 2, 128, 2048], BF16, kind="ExternalInput"
        ),
        "y": nc.dram_tensor("y", [BPC, 2, 128, N], F32, kind="ExternalOutput"),
    }
    d = {k: (v.ap() if hasattr(v, "ap") else v) for k, v in d.items()}
    with tile.TileContext(nc) as tc:
        with (
            tc.tile_pool(name="main", bufs=1) as pool,
            tc.tile_pool(name="psum", bufs=2, space="PSUM") as psum_pool,
            tc.tile_pool(name="dram", bufs=2, space="DRAM") as dram_pool,
        ):
            d["_pool"] = pool
            d["_psum_pool"] = psum_pool
            d["_dram_pool"] = dram_pool
            build_body(nc, tc, d)
    nc.compile()
    return nc


def _prep_inputs(inputs):
    f = lambda k: np.asarray(inputs[k], np.float32)
    x = f("x")
    sq = f("gq") / np.sqrt(f("vq") + EPS)
    sk = f("gk") / np.sqrt(f("vk") + EPS)
    sv = f("gv") / np.sqrt(f("vv") + EPS)
    so = f("go") / np.sqrt(f("vo") + EPS)
    Wq = f("wq") * sq[:, None] * SCALE
    bq = (f("bq") - f("mq") * sq) * SCALE
    Wk = f("wk") * sk[:, None]
    bk = f("bk") - f("mk") * sk
    Wv = f("wv") * sv[:, None]
    bv = f("bv") - f("mv") * sv
    Wo = f("w_out") * so[:, None] / 6.0
    bo = so * f("b_out") + f("bo") - f("mo") * so

    emb = f("emb")
    pos = np.asarray(inputs["pos_indices"], np.int64)
    EB = np.exp(emb[pos].transpose(2, 0, 1) / SCALE)  # [H, N(j), N(i)]

    bf = ml_dtypes.bfloat16
    wpack = np.concatenate(
        [
            Wq.T.reshape(2, 128, 512).transpose(1, 0, 2).reshape(128, 1024),
            Wk.T.reshape(2, 128, 512).transpose(1, 0, 2).reshape(128, 1024),
            Wv.T.reshape(2, 128, 512).transpose(1, 0, 2).reshape(128, 1024),
            Wo.T.reshape(4, 128, 256).transpose(1, 0, 2).reshape(128, 1024),
        ],
        axis=1,
    )
    bpack = np.concatenate(
        [bq.reshape(4, 128).T, bk.reshape(4, 128).T, bo.reshape(2, 128).T], axis=1
    )
    # EB4: [pair, jpair, ic, 128(jrow), 2048]
    # free dim: [j=2jp: h-half(512) | h'-half(512) | j=2jp+1: h-half | h'-half]
    EBj = EB.reshape(H, 8, 128, 2, 512)  # [h, jtile, jrow, ic, i-half]
    EB3 = np.stack(
        [np.concatenate([EBj[0::2, :, :, ic], EBj[1::2, :, :, ic]], axis=3)
         for ic in range(2)],
        axis=2,
    )  # [pair, jt, ic, 128(jrow), 1024]
    EB4 = (
        EB3.reshape(NPAIR, 4, 2, 2, 128, 1024)  # [pair, jp, jh, ic, jrow, 1024]
        .transpose(0, 1, 3, 4, 2, 5)            # [pair, jp, ic, jrow, jh, 1024]
        .reshape(NPAIR, 4, 2, 128, 2048)
    )
    shared = {
        "wpack": np.ascontiguousarray(wpack).astype(bf),
        "bpack": np.ascontiguousarray(bpack),
        "bv": bv.reshape(1, 512).astype(bf),
        "eb4": np.ascontiguousarray(EB4).astype(bf),
    }
    x_dev = x.reshape(B, 2, 128, N).astype(bf)
    in_maps = [
        dict(shared, x=np.ascontiguousarray(x_dev[c * BPC : (c + 1) * BPC]))
        for c in range(NCORES)
    ]
    return in_maps


def kernel(**inputs):
    nc = build_nc()
    in_maps = _prep_inputs(inputs)
    res = run_bass_kernel_spmd(nc, in_maps, core_ids=list(range(NCORES)))
    y = np.concatenate([r["y"].reshape(BPC, DIM, 32, 32) for r in res.results], axis=0)
    return y.astype(np.float32)


def _install_ntff_hook():
    """The image's antenv lacks axon_hooks; synthesize it so trace=True works."""
    import types

    try:
        from antenv.axon_hooks import get_axon_ntff_profile_hook  # noqa: F401

        return
    except ImportError:
        pass
    import antenv
    from trn_agent_boot.trn_boot import _ntff_profile_via_ctypes

    mod = types.ModuleType("antenv.axon_hooks")
    mod._hook = _ntff_profile_via_ctypes("/opt/axon/libaxon_pjrt.so")
    mod.get_axon_ntff_profile_hook = lambda: mod._hook
    mod.set_axon_ntff_profile_hook = lambda h: setattr(mod, "_hook", h)
    sys.modules["antenv.axon_hooks"] = mod
    antenv.axon_hooks = mod

    # no artifact bucket in this container; neuter the upload
    import concourse.bass_utils as bu

    bu.upload_artifacts = lambda tmpdir: f"local:{tmpdir}"


def run_traced(inputs, tmpdir=None):
    """Like kernel() but with NTFF tracing; returns (y, BassKernelResults)."""
    _install_ntff_hook()
    nc = build_nc()
    in_maps = _prep_inputs(inputs)
    res = run_bass_kernel_spmd(
        nc, in_maps, core_ids=list(range(NCORES)), trace=True, tmpdir=tmpdir
    )
    y = np.concatenate([r["y"].reshape(BPC, DIM, 32, 32) for r in res.results], axis=0)
    return y.astype(np.float32), res



# revision 28
# speedup vs baseline: 1.1318x; 1.1318x over previous
"""Trainium2 Bass kernel for BoT-style attention (nn_Attention_20968030339767).

Data-parallel over batch: 16 batches -> 2 per NeuronCore, 8 cores, no
collectives.  All BN folding / bias-table exponentiation happens on host;
the device runs projections + attention + hardswish + output projection.

Math (per batch b):
  q = BN(Wq x), k = BN(Wk x), v = BN(Wv x)          (1x1 conv == channel matmul)
  logits = SCALE*(q.k) + emb[pos]/SCALE
  attn   = softmax(logits); out = attn @ v
  hs     = hardswish(out); y = BN(W_out hs + b_out)

Device-side formulation (per core, all matmuls bf16, PSUM f32):
  qT = Wq'' x   [512, 1024]  (Wq'' = SCALE*diag(sq)*Wq; bias bq'' at evac)
  kT = Wk'  x   [512, 1024]
  vT = x^T Wv'^T + bv'       [1024, 8, 65]  (per-head blocks: 64 v-cols + ones)

  Attention runs per (head-pair p, batch b, i-half ic).  The channel layout
  puts head 2p at SBUF partitions 0-63 and head 2p+1 at 64-127 of the m2=p
  projection tile, so the two heads' QK^T matmuls (K=64 contraction) run
  CONCURRENTLY in the PE array via row-group tiling (tile_position derives
  from base_partition 0/64), writing one 2-bank PSUM tile
  st2 [128 j, 1024] = [h (512 i) | h' (512 i)] per (j, ic).  Then per j-pair:
    exp(st2_j), exp(st2_j+1) -> halves of one p2 [128, 2048] bf16 tile
    p2 *= EB4[p, jp, ic]         one in-place DVE instr, 2x bf16 mode
    utpair[:, 0:512]    += vT_aug_h^T  @ p2-slices   (row 64 = sumexp)
    utpair[:, 512:1024] += vT_aug_h'^T @ p2-slices
  since vT carries bv': UT[0:64] = U_raw + bv'*sumexp.
  Normalize chain per ic (software-pipelined: emitted after the NEXT ic's
  first j-pair so the FIFO engine queues never stall on it):
    sums copy -> reciprocal_approx_fast (~51 ULP, plenty) -> DRAM round-trip
    broadcast f32 [64, N] -> u2 quadrant = UT[0:64] * bcast  (reads PSUM,
    evacuates the UT banks; = U/sum + bv' after normalize)
  hardswish in 2 DVE ops: t1 = max(u2+3, 0);  hs = min(t1, 6) * u2
  (the /6 is folded into Wo'')
  y = Wo'' hs + bo''   (Wo'' = diag(so)*W_out/6)

Scheduling notes (the exp chain is the ~134us floor; everything is built to
keep the scalar engine streaming):
  - PSUM (8 banks exactly): tag "st2" [128,1024] bufs=2 (4 banks) is
    attention-EXCLUSIVE so no projection tile ever FIFO-chains ahead of an
    exp; tag "ut" bufs=2 (4 banks) holds the utpair accumulators plus the
    qp/kp/vp/yp projection tiles (PV-side stalls absorb via the deep p2 ring).
  - PV matmuls are emitted TWO j-pairs late via a rolling queue that
    crosses ic/unit boundaries: at ~94% vector utilization each eb-multiply
    lands ~1 slot late, so lag-1 PVs were mul-paced and dragged the next QKs
    (and thus exps) with them through the in-order tensor queue.
  - Projections for pair p+1 and its EB fetch are emitted one unit early,
    woven between attention units.
  - gpsimd is useless here: ~2.9us/instr tensor ops and no PSUM port.

Softmax max-subtraction is skipped: |logits| <= ~7 here, exp is safe in f32.
No collectives; cores are fully independent (batch-parallel).
"""

import sys
import functools

import numpy as np

sys.path.insert(0, "/opt/trn_rl_repo")

import ml_dtypes  # noqa: E402

from concourse import bacc, mybir  # noqa: E402
import concourse.bass as bass  # noqa: E402
import concourse.tile as tile  # noqa: E402
from concourse.bass_utils import run_bass_kernel_spmd  # noqa: E402

BF16 = mybir.dt.bfloat16
F32 = mybir.dt.float32
Af = mybir.ActivationFunctionType
Op = mybir.AluOpType

B, DIM, H, DK, N = 16, 256, 8, 64, 1024
INK = H * DK  # 512
SCALE = DK ** -0.5
EPS = 1e-5
NCORES = 8
BPC = B // NCORES  # batches per core
NPAIR = H // 2  # head pairs
# which jp's eb-multiply runs on gpsimd instead of the (bottleneck) DVE.
# jp3 must stay on the DVE: its PV pops right before the part1 drain, so it
# cannot take the extra lag slot that slow (~4.4us) gpsimd multiplies need.
EB_GP_JP = frozenset({1})


def build_body(nc, tc, d):
    """Emit the whole per-core program inside a TileContext."""
    ts = bass.ts

    _n = [0]
    pool = d["_pool"]

    def T(shape, dtype, **kw):
        if "name" not in kw:
            kw["name"] = f"{kw.get('tag', 't')}_{_n[0]}"
            _n[0] += 1
        space = kw.pop("space", None)
        if space == "PSUM":
            p = d["_psum_pool"]
        elif space == "DRAM":
            p = d["_dram_pool"]
        else:
            p = pool
        return p.tile(shape, dtype, **kw)

    # ---- persistent SBUF tensors (weights packed into one DMA) -----------
    wpk = T([128, 4096], BF16, tag="wpk", bufs=1)
    bpk = T([128, 10], F32, tag="bpk", bufs=1)
    bv_bc = T([128, 512], BF16, tag="bv_bc", bufs=1)
    x_s = []
    for b in range(BPC):
        xt = T([128, 2, N], BF16, tag="x", bufs=BPC, name=f"x{b}")
        x_s.append(xt)
    # prologue DMA order = the first QK pair's exact dependency chain first:
    # x0, biases, then ONLY the m=0 columns of wq/wk (4x32KB), so the first
    # projection starts ~8us earlier than with full-tensor fetches
    nc.sync.dma_start(x_s[0][:], d["x"][0].rearrange("k p n -> p k n"))
    nc.sync.dma_start(bpk[:], d["bpack"][:])
    for c0 in (0, 512, 1024, 1536):
        nc.sync.dma_start(wpk[:, c0 : c0 + 128], d["wpack"][:, c0 : c0 + 128])
    nc.sync.dma_start(x_s[1][:], d["x"][1].rearrange("k p n -> p k n"))
    for c0 in (0, 512, 1024, 1536):
        nc.sync.dma_start(wpk[:, c0 + 128 : c0 + 512],
                          d["wpack"][:, c0 + 128 : c0 + 512])
    nc.sync.dma_start(wpk[:, 2048:3072], d["wpack"][:, 2048:3072])
    # bv broadcast to all 128 partitions (token rows) via step-0 DMA
    nc.sync.dma_start(bv_bc[:], d["bv"].broadcast_to([128, 512]))
    nc.sync.dma_start(wpk[:, 3072:4096], d["wpack"][:, 3072:4096])
    wq_s = wpk[:].rearrange("p (w k o) -> p w k o", w=4, k=2)[:, 0]
    wk_s = wpk[:].rearrange("p (w k o) -> p w k o", w=4, k=2)[:, 1]
    wv_s = wpk[:].rearrange("p (w k o) -> p w k o", w=4, k=2)[:, 2]
    wo_s = wpk[:, 3072:4096].rearrange("p (k o) -> p k o", k=4)
    bq_s = bpk[:, 0:4]
    bk_s = bpk[:, 4:8]
    bo_s = bpk[:, 8:10]

    q_s = [[None] * 4 for _ in range(BPC)]
    k_s = [[None] * 4 for _ in range(BPC)]
    v_s = [[None] * 8 for _ in range(BPC)]
    hs_s = [[None] * 4 for _ in range(BPC)]

    # ---- projection pieces (emitted interleaved between attention units) --
    def proj_q(b, m):
        qp = T([128, N], F32, space="PSUM", tag="ut", bufs=2, name=f"qp{b}{m}")
        for ic in range(2):
            for kc in range(2):
                nc.tensor.matmul(
                    qp[:, ts(ic, 512)],
                    wq_s[:, kc, ts(m, 128)],
                    x_s[b][:, kc, ts(ic, 512)],
                    start=(kc == 0),
                    stop=(kc == 1),
                )
        qt = T([128, N], BF16, tag="q", bufs=6, name=f"q{b}{m}")
        nc.vector.tensor_scalar(qt[:], qp[:], bq_s[:, m : m + 1], None, Op.add)
        q_s[b][m] = qt

    def proj_k(b, m):
        kp = T([128, N], F32, space="PSUM", tag="ut", bufs=2, name=f"kp{b}{m}")
        for ic in range(2):
            for kc in range(2):
                nc.tensor.matmul(
                    kp[:, ts(ic, 512)],
                    wk_s[:, kc, ts(m, 128)],
                    x_s[b][:, kc, ts(ic, 512)],
                    start=(kc == 0),
                    stop=(kc == 1),
                )
        kt = T([128, N], BF16, tag="k", bufs=6, name=f"k{b}{m}")
        nc.vector.tensor_scalar(kt[:], kp[:], bk_s[:, m : m + 1], None, Op.add)
        k_s[b][m] = kt

    def proj_qk(b, m):
        proj_q(b, m)
        proj_k(b, m)

    def proj_v(b):
        for t in range(8):
            vp = T([128, 512], F32, space="PSUM", tag="ut", bufs=2, name=f"vp{b}{t}")
            for kc in range(2):
                nc.tensor.matmul(
                    vp[:],
                    x_s[b][:, kc, ts(t, 128)],
                    wv_s[:, kc, :],
                    start=(kc == 0),
                    stop=(kc == 1),
                )
            # evac + bv'' add; bias rides into UT as bv*sumexp
            vt = T([128, 8, 65], BF16, tag="v", bufs=8 * BPC, name=f"v{b}{t}")
            nc.gpsimd.memset(vt[:, :, 64:65], 1.0)
            nc.vector.tensor_add(
                vt[:, :, 0:64],
                vp[:].rearrange("p (h e) -> p h e", e=64),
                bv_bc[:].rearrange("p (h e) -> p h e", e=64),
            )
            v_s[b][t] = vt

    def yproj(b):
        for m in range(2):
            for t2 in range(2):
                yp = T([128, 512], F32, space="PSUM", tag="ut", bufs=2,
                       name=f"yp{b}{m}{t2}")
                for kc in range(4):
                    nc.tensor.matmul(
                        yp[:],
                        wo_s[:, kc, ts(m, 128)],
                        hs_s[b][kc][:, ts(t2, 512)],
                        start=(kc == 0),
                        stop=(kc == 3),
                    )
                ys = T([128, 512], F32, tag="y", bufs=3, name=f"ys{b}{m}{t2}")
                nc.scalar.activation(ys[:], yp[:], Af.Identity, bias=bo_s[:, m : m + 1])
                nc.sync.dma_start(d["y"][b, m, :, ts(t2, 512)], ys[:])

    # ---- attention unit: head-pair (2p, 2p+1) of batch b ------------------
    # The recip/normalize chain for each ic-half is emitted AFTER the next
    # ic's first j-pair so the FIFO engine queues never stall on it.
    pend = [None, None]
    pvq = []

    def drain(slot):
        if pend[slot] is not None:
            pend[slot]()
            pend[slot] = None

    def unit(p, b, ebt, pieces=(), ebq=()):
        u2 = T([128, N], BF16, tag="u2", bufs=2, name=f"u2{p}{b}")

        def mk_chain(ut, ic, final):
            # split in two so the vector-FIFO insert is spread over two drain
            # slots (and the broadcast DMA has landed before part2 needs it)
            bcast = T([64, N], F32, tag="bcast", bufs=2, name=f"bc{p}{b}{ic}")

            def part1():
                # softmax denominators: UT row 64 = [sum_h | sum_h'].
                # approx-recip (~51 ULP, plenty) straight off the PSUM sums
                # row, then a DRAM round-trip broadcast to [64, N] f32 (DMA
                # engines are idle; gpsimd is reserved for eb-multiplies).
                sums = T([1, N], F32, tag="sums", bufs=1, name=f"sm{p}{b}{ic}")
                nc.vector.tensor_copy(sums[:], ut[64:65, :])
                rc = T([1, N], F32, tag="rc", bufs=1, name=f"rc{p}{b}{ic}")
                nc.vector.reciprocal_approx_fast(rc[:], sums[:])
                # broadcast via DRAM round-trip (SBUF-src broadcast APs are
                # rejected by the DMA lowering)
                dr = T([1, N], F32, space="DRAM", tag="dr", bufs=2,
                       name=f"dr{p}{b}{ic}")
                nc.sync.dma_start(dr[:], rc[:])
                nc.sync.dma_start(bcast[:], dr[:].broadcast_to([64, N]))

            def part2():
                # normalize straight out of PSUM (also evacuates the UT banks)
                nc.vector.tensor_mul(
                    u2[0:64, ts(ic, 512)], ut[0:64, 0:512], bcast[:, 0:512]
                )
                nc.vector.tensor_mul(
                    u2[64:128, ts(ic, 512)], ut[0:64, 512:1024], bcast[:, 512:1024]
                )
                if final:
                    # hardswish: hs = min(max(u2+3,0),6) * u2  (/6 in Wo'')
                    # (dual-alu-op instrs don't compile on gpsimd, so DVE)
                    t1 = T([128, N], BF16, tag="t1", bufs=2, name=f"t1{p}{b}")
                    nc.vector.tensor_scalar(t1[:], u2[:], 3.0, 0.0, Op.add, Op.max)
                    hst = T([128, N], BF16, tag="hs", bufs=4 * BPC, name=f"hs{p}{b}")
                    nc.vector.scalar_tensor_tensor(
                        hst[:], t1[:], 6.0, u2[:], Op.min, Op.mult
                    )
                    hs_s[b][p] = hst
            return part1, part2

        for ic in range(2):
            ut = T([65, N], F32, space="PSUM", tag="ut", bufs=2, name=f"ut{p}{b}{ic}")

            def pv(jp, p2, ut=ut):
                for jh in range(2):
                    j = 2 * jp + jh
                    nc.tensor.matmul(
                        ut[:, 0:512],
                        v_s[b][j][:, 2 * p, :],
                        p2[:, 1024 * jh : 1024 * jh + 512],
                        start=(j == 0),
                        stop=(j == 7),
                    )
                    nc.tensor.matmul(
                        ut[:, 512:1024],
                        v_s[b][j][:, 2 * p + 1, :],
                        p2[:, 1024 * jh + 512 : 1024 * jh + 1024],
                        start=(j == 0),
                        stop=(j == 7),
                    )

            for jp in range(4):
                # j-pair shares one [128, 2048] p2 (exp writes halves, eb-mul
                # in place) so the multiply runs as one DVE instr in 2x mode;
                # PV for pair jp is emitted one pair late via a rolling queue
                # that crosses ic/unit boundaries, so neither a multiply nor a
                # PV burst ever stalls the next QKs (in-order tensor queue)
                p2 = T([128, 2048], BF16, tag="ep", bufs=10, name=f"ep{p}{b}{ic}{jp}")
                for jh in range(2):
                    j = 2 * jp + jh
                    st2 = T([128, N], F32, space="PSUM", tag="st2", bufs=2,
                            name=f"st{p}{b}{ic}{j}")
                    # the two heads' QK^T run concurrently in the PE array
                    # (row groups 0/64 via base_partition -> different banks)
                    nc.tensor.matmul(
                        st2[:, 0:512],
                        k_s[b][p][0:64, ts(j, 128)],
                        q_s[b][p][0:64, ts(ic, 512)],
                        start=True,
                        stop=True,
                    )
                    nc.tensor.matmul(
                        st2[:, 512:1024],
                        k_s[b][p][64:128, ts(j, 128)],
                        q_s[b][p][64:128, ts(ic, 512)],
                        start=True,
                        stop=True,
                    )
                    nc.scalar.activation(p2[:, ts(jh, 1024)], st2[:], Af.Exp)
                # eb-multiply first: it must never queue behind the chain ops
                # the drains below inject into the same (in-order) DVE queue.
                # gpsimd (otherwise idle) takes a subset of the 64 multiplies
                # to relieve the DVE bottleneck.
                if jp in EB_GP_JP:
                    nc.gpsimd.tensor_mul(p2[:], p2[:], ebt[:, jp, ic, :])
                else:
                    nc.vector.tensor_mul(p2[:], p2[:], ebt[:, jp, ic, :])
                # PV lag is TWO pairs (THREE for gpsimd-multiplied pairs: the
                # gp multiply takes ~4.4us vs ~1.2 on DVE, so its PVs get one
                # extra emission slot of slack, deferred at most once).  At
                # jp==1 ALL of the previous ic's PVs are force-flushed so the
                # part1 drain never reads the sums row before its final
                # accumulation has been emitted.
                if jp == 1:
                    while len(pvq) > 1:
                        f, a, t, _ = pvq.pop(0)
                        f(a, t)
                    drain(0)
                elif len(pvq) >= 2:
                    if pvq[0][1] in EB_GP_JP and len(pvq) == 2 and not pvq[0][3]:
                        pvq[0][3] = True  # defer this gp-paced PV once
                    else:
                        f, a, t, _ = pvq.pop(0)
                        f(a, t)
                if jp == 3:
                    # ~2.3us after part1: its broadcast has landed; the u-muls
                    # must beat PVs two blocks out to the UT banks
                    drain(1)
                pvq.append([pv, jp, p2, False])
                if jp == 3 and pieces:
                    # a "ut"-ring slot was just scheduled for release by the
                    # part2 drain above: the next projection piece can cycle
                    # the ring with only ~1us of tensor-head wait
                    pieces.pop(0)()
                if ebq:
                    # next pair's EB table arrives one 512KB chunk per slot so
                    # it never head-of-line blocks the bcast round-trips that
                    # share the sync DMA ring
                    ebq.pop(0)()
            pend[0], pend[1] = mk_chain(ut, ic, final=(ic == 1))

    def fetch_eb(p, chunked=False):
        """Allocate pair p's EB tile; either fetch now or return chunk DMAs."""
        ebt = T([128, 4, 2, 2048], BF16, tag="eb", bufs=2, name=f"eb{p}")
        chunks = [
            (lambda jp=jp, c=c: nc.sync.dma_start(ebt[:, jp, c],
                                                  d["eb4"][p, jp, c]))
            for jp in range(4) for c in range(2)
        ]
        if not chunked:
            for ch in chunks:
                ch()
            chunks = []
        return ebt, chunks

    # ---- interleaved emission --------------------------------------------
    # unit order: (p0,b0),(p0,b1),(p1,b0),... so each pair's EB tile serves
    # both batches; proj pieces are woven in so no PSUM-ring FIFO chains a
    # whole phase ahead of the attention stream.
    ebt, _ = fetch_eb(0)
    proj_qk(0, 0)
    proj_v(0)
    proj_qk(1, 0)
    proj_v(1)
    next_eb = None
    pieces = []
    ebq = []
    for p in range(NPAIR):
        for b in range(BPC):
            if b == 0 and p + 1 < NPAIR:
                # stage pair p+1's projections as 4 pieces popped one-by-one
                # right after each part2 drain (2 "ut"-slot release windows
                # per unit) so proj matmuls never bunch up ahead of QKs
                pieces += [
                    lambda m=p + 1: proj_q(0, m),
                    lambda m=p + 1: proj_k(0, m),
                    lambda m=p + 1: proj_q(1, m),
                    lambda m=p + 1: proj_k(1, m),
                ]
            if b == 1 and p + 1 < NPAIR:
                next_eb, ebq[:] = fetch_eb(p + 1, chunked=True)
            unit(p, b, ebt, pieces, ebq)
        if next_eb is not None:
            ebt = next_eb
            next_eb = None
    while pvq:
        f, a, t, _ = pvq.pop(0)
        f(a, t)
    # batch 0's hardswish tiles were all drained mid-stream, so its output
    # projection is independent of the final unit's pending chain: emit it
    # first to overlap that chain's DMA round-trip
    yproj(0)
    drain(0)
    drain(1)
    yproj(1)


@functools.cache
def build_nc():
    nc = bacc.Bacc(
        "TRN2",
        target_bir_lowering=False,
        debug=False,
        enable_asserts=False,
        num_devices=NCORES,
    )
    d = {
        "x": nc.dram_tensor("x", [BPC, 2, 128, N], BF16, kind="ExternalInput"),
        "wpack": nc.dram_tensor("wpack", [128, 4096], BF16, kind="ExternalInput"),
        "bpack": nc.dram_tensor("bpack", [128, 10], F32, kind="ExternalInput"),
        "bv": nc.dram_tensor("bv", [1, 512], BF16, kind="ExternalInput"),
        "eb4": nc.dram_tensor(
            "eb4", [NPAIR, 4, 2, 128, 2048], BF16, kind="ExternalInput"
        ),
        "y": nc.dram_tensor("y", [BPC, 2, 128, N], F32, kind="ExternalOutput"),
    }
    d = {k: (v.ap() if hasattr(v, "ap") else v) for k, v in d.items()}
    with tile.TileContext(nc) as tc:
        with (
            tc.tile_pool(name="main", bufs=1) as pool,
            tc.tile_pool(name="psum", bufs=2, space="PSUM") as psum_pool,
            tc.tile_pool(name="dram", bufs=2, space="DRAM") as dram_pool,
        ):
            d["_pool"] = pool
            d["_psum_pool"] = psum_pool
            d["_dram_pool"] = dram_pool
            build_body(nc, tc, d)
    nc.compile()
    return nc


def _prep_inputs(inputs):
    f = lambda k: np.asarray(inputs[k], np.float32)
    x = f("x")
    sq = f("gq") / np.sqrt(f("vq") + EPS)
    sk = f("gk") / np.sqrt(f("vk") + EPS)
    sv = f("gv") / np.sqrt(f("vv") + EPS)
    so = f("go") / np.sqrt(f("vo") + EPS)
    Wq = f("wq") * sq[:, None] * SCALE
    bq = (f("bq") - f("mq") * sq) * SCALE
    Wk = f("wk") * sk[:, None]
    bk = f("bk") - f("mk") * sk
    Wv = f("wv") * sv[:, None]
    bv = f("bv") - f("mv") * sv
    Wo = f("w_out") * so[:, None] / 6.0
    bo = so * f("b_out") + f("bo") - f("mo") * so

    emb = f("emb")
    pos = np.asarray(inputs["pos_indices"], np.int64)
    EB = np.exp(emb[pos].transpose(2, 0, 1) / SCALE)  # [H, N(j), N(i)]

    bf = ml_dtypes.bfloat16
    wpack = np.concatenate(
        [
            Wq.T.reshape(2, 128, 512).transpose(1, 0, 2).reshape(128, 1024),
            Wk.T.reshape(2, 128, 512).transpose(1, 0, 2).reshape(128, 1024),
            Wv.T.reshape(2, 128, 512).transpose(1, 0, 2).reshape(128, 1024),
            Wo.T.reshape(4, 128, 256).transpose(1, 0, 2).reshape(128, 1024),
        ],
        axis=1,
    )
    bpack = np.concatenate(
        [bq.reshape(4, 128).T, bk.reshape(4, 128).T, bo.reshape(2, 128).T], axis=1
    )
    # EB4: [pair, jpair, ic, 128(jrow), 2048]
    # free dim: [j=2jp: h-half(512) | h'-half(512) | j=2jp+1: h-half | h'-half]
    EBj = EB.reshape(H, 8, 128, 2, 512)  # [h, jtile, jrow, ic, i-half]
    EB3 = np.stack(
        [np.concatenate([EBj[0::2, :, :, ic], EBj[1::2, :, :, ic]], axis=3)
         for ic in range(2)],
        axis=2,
    )  # [pair, jt, ic, 128(jrow), 1024]
    EB4 = (
        EB3.reshape(NPAIR, 4, 2, 2, 128, 1024)  # [pair, jp, jh, ic, jrow, 1024]
        .transpose(0, 1, 3, 4, 2, 5)            # [pair, jp, ic, jrow, jh, 1024]
        .reshape(NPAIR, 4, 2, 128, 2048)
    )
    shared = {
        "wpack": np.ascontiguousarray(wpack).astype(bf),
        "bpack": np.ascontiguousarray(bpack),
        "bv": bv.reshape(1, 512).astype(bf),
        "eb4": np.ascontiguousarray(EB4).astype(bf),
    }
    x_dev = x.reshape(B, 2, 128, N).astype(bf)
    in_maps = [
        dict(shared, x=np.ascontiguousarray(x_dev[c * BPC : (c + 1) * BPC]))
        for c in range(NCORES)
    ]
    return in_maps


def kernel(**inputs):
    nc = build_nc()
    in_maps = _prep_inputs(inputs)
    res = run_bass_kernel_spmd(nc, in_maps, core_ids=list(range(NCORES)))
    y = np.concatenate([r["y"].reshape(BPC, DIM, 32, 32) for r in res.results], axis=0)
    return y.astype(np.float32)


def _install_ntff_hook():
    """The image's antenv lacks axon_hooks; synthesize it so trace=True works."""
    import types

    try:
        from antenv.axon_hooks import get_axon_ntff_profile_hook  # noqa: F401

        return
    except ImportError:
        pass
    import antenv
    from trn_agent_boot.trn_boot import _ntff_profile_via_ctypes

    mod = types.ModuleType("antenv.axon_hooks")
    mod._hook = _ntff_profile_via_ctypes("/opt/axon/libaxon_pjrt.so")
    mod.get_axon_ntff_profile_hook = lambda: mod._hook
    mod.set_axon_ntff_profile_hook = lambda h: setattr(mod, "_hook", h)
    sys.modules["antenv.axon_hooks"] = mod
    antenv.axon_hooks = mod

    # no artifact bucket in this container; neuter the upload
    import concourse.bass_utils as bu

    bu.upload_artifacts = lambda tmpdir: f"local:{tmpdir}"


def run_traced(inputs, tmpdir=None):
    """Like kernel() but with NTFF tracing; returns (y, BassKernelResults)."""
    _install_ntff_hook()
    nc = build_nc()
    in_maps = _prep_inputs(inputs)
    res = run_bass_kernel_spmd(
        nc, in_maps, core_ids=list(range(NCORES)), trace=True, tmpdir=tmpdir
    )
    y = np.concatenate([r["y"].reshape(BPC, DIM, 32, 32) for r in res.results], axis=0)
    return y.astype(np.float32), res

